# revision 1
# baseline (speedup 1.0000x reference)
"""Correspondence-loss kernel for TRN2, 8 NeuronCores, data-parallel over batch.

Contract: kernel(**inputs) takes the FULL unsharded inputs (numpy) and
returns the FULL scalar output, matching reference.reference().

Design
------
Per core i (of 8): batches [2i, 2i+1].
Host precomputes, per core:
  - flat gather row indices into the core's [8192, 768] feature shards
    (pixel->patch indexing + clamp is tiny int math on [B,N] arrays)
  - valid mask as f32, laid out [128 partitions, 4 column-tiles]
Device per core:
  - 8x indirect DMA gathers: 128 rows x 3072B each (the only significant
    HBM traffic: 2 * 512 * 3072B = 3.1 MB/core -> ~9us at 358 GB/s)
  - DVE tensor_tensor_reduce: dot(s,t) and sum(s^2) fused product+row-reduce
  - ACT Square activation with accum_out: sum(t^2)
  - tiny [128,4] epilogue: cos = dot / sqrt(max(ss*tt, 1e-16)), out = cos*mask
Host: loss = (n_valid - sum(out)) / max(n_valid, 1)   [since (1-cos)*m sums
to sum(m) - sum(cos*m), and n_valid is known on host from the mask]
"""

import os
import sys

import numpy as np

for _p in ("/opt/trn_rl_repo",):
    if os.path.isdir(_p) and _p not in sys.path:
        sys.path.insert(0, _p)

from concourse import bass, mybir, tile  # noqa: E402
from concourse.bass import IndirectOffsetOnAxis  # noqa: E402
from concourse.bass_utils import run_bass_kernel_spmd  # noqa: E402

M = 8                 # cores
B, H, W, D, N = 16, 64, 64, 768, 256
BPC = B // M          # batches per core
KPC = BPC * N         # keypoints per core
P = 128               # SBUF partitions
C = KPC // P          # column tiles per core (4)
ROWS = BPC * H * W    # feature rows per core (8192)
F32 = mybir.dt.float32
I32 = mybir.dt.int32

LAST_RUN = None       # BassKernelResults of the most recent run (for test.py)


def build_nc(gather_plan=None, meta_engine="gpsimd", junk_bufs=2,
             act_ops=("tt0", "tt1", "tt2", "tt3"),
             out_engine="sync", split3=False) -> bass.Bass:
    # meta layout (int32 [P, 12]): cols 0-3 src row idx (per column tile),
    # cols 4-7 tgt row idx, cols 8-11 valid mask as f32 bits.
    nc = bass.Bass()
    src = nc.declare_dram_parameter("src_feat", [ROWS, D], F32, isOutput=False)
    tgt = nc.declare_dram_parameter("tgt_feat", [ROWS, D], F32, isOutput=False)
    meta_d = nc.declare_dram_parameter("meta", [P, 12], I32, isOutput=False)
    out_d = nc.declare_dram_parameter("out", [P, C], F32, isOutput=True)

    mult = mybir.AluOpType.mult
    Square = mybir.ActivationFunctionType.Square

    if gather_plan is None:
        # (kind, first column tile, n tiles): src before tgt, tiles 0,1
        # batched, so compute streams behind the DMA and post-last-byte
        # exposure is just {dot3, tt3} + epilogue
        gather_plan = [("s", 0, 2), ("t", 0, 2), ("s", 2, 1), ("t", 2, 1),
                       ("s", 3, 1), ("t", 3, 1)]
    if split3:
        # last tgt tile arrives in halves so the final dot/tt passes are
        # half-length: shorter exposure after the last gathered byte
        gather_plan = [g for g in gather_plan if g != ("t", 3, 1)]

    with tile.TileContext(nc) as tc:
        with (
            tc.tile_pool(name="big", bufs=1) as big,
            tc.tile_pool(name="small", bufs=1) as small,
            tc.tile_pool(name="junk", bufs=junk_bufs) as junkp,
        ):
            meta = small.tile([P, 12], I32)
            meta_eng = nc.sync if meta_engine == "sync" else nc.gpsimd
            meta_eng.dma_start(out=meta[:], in_=meta_d[:])
            maskt = meta[:, 8:12].bitcast(F32)

            # warm the ACT function table (Square/Sqrt set) while DMAs run
            warm = small.tile([P, 1], F32)
            nc.scalar.activation(out=warm[:], in_=maskt[:, 0:1], func=Square)

            dott = small.tile([P, C], F32)
            sst = small.tile([P, C], F32)
            ttt = small.tile([P, C], F32)

            sl: dict = {}
            tl: dict = {}
            avail: dict = {}   # op name -> gather order index it needs
            for gi, (kind, c0, n) in enumerate(gather_plan):
                g = big.tile([P, n * D], F32, tag=f"g{gi}")
                table = src if kind == "s" else tgt
                col = c0 + (0 if kind == "s" else 4)
                nc.gpsimd.indirect_dma_start(
                    out=g[:],
                    out_offset=None,
                    in_=table[:],
                    in_offset=IndirectOffsetOnAxis(ap=meta[:, col : col + n], axis=0),
                )
                for j in range(n):
                    (sl if kind == "s" else tl)[c0 + j] = g[:, j * D : (j + 1) * D]
                    avail[("ss" if kind == "s" else "tt") + str(c0 + j)] = gi
            for c in range(C):
                if f"ss{c}" in avail and f"tt{c}" in avail:
                    avail[f"dot{c}"] = max(avail[f"ss{c}"], avail[f"tt{c}"])

            def emit(op):
                c = int(op[-1])
                if op.startswith("dot"):
                    j = junkp.tile([P, D], F32, tag="dve_junk")
                    nc.vector.scalar_tensor_tensor(
                        out=j[:], in0=sl[c], scalar=1.0, in1=tl[c],
                        op0=mult, op1=mult, accum_out=dott[:, c : c + 1],
                    )
                    return
                src_ap = sl[c] if op.startswith("ss") else tl[c]
                acc = (sst if op.startswith("ss") else ttt)[:, c : c + 1]
                if op in act_ops:
                    j = junkp.tile([P, D], F32, tag="act_junk")
                    nc.scalar.activation(out=j[:], in_=src_ap, func=Square,
                                         accum_out=acc)
                else:
                    j = junkp.tile([P, D], F32, tag="dve_junk")
                    nc.vector.scalar_tensor_tensor(
                        out=j[:], in0=src_ap, scalar=1.0, in1=src_ap,
                        op0=mult, op1=mult, accum_out=acc,
                    )

            last = C - 1
            ops = [f"{k}{c}" for c in range(C) for k in ("ss", "tt", "dot")]
            if split3:
                ops = [o for o in ops if o not in (f"tt{last}", f"dot{last}")]
                avail[f"ss{last}"] = len(gather_plan) - 1
            for op in sorted(ops, key=lambda o: (avail.get(o, 99), o.startswith("dot"))):
                emit(op)

            if split3:
                Dh = D // 2
                t3a = big.tile([P, Dh], F32)
                t3b = big.tile([P, Dh], F32)
                for half, off in ((t3a, 0), (t3b, Dh)):
                    nc.gpsimd.indirect_dma_start(
                        out=half[:], out_offset=None, in_=tgt[:],
                        in_offset=IndirectOffsetOnAxis(
                            ap=meta[:, 4 + last : 5 + last], axis=0),
                        element_offset=off,
                    )
                dh = small.tile([P, 2], F32)
                th = small.tile([P, 2], F32)
                for j, (half, off) in enumerate(((t3a, 0), (t3b, Dh))):
                    jt = junkp.tile([P, Dh], F32, tag="act_junk")
                    nc.scalar.activation(out=jt[:], in_=half[:], func=Square,
                                         accum_out=th[:, j : j + 1])
                    jd = junkp.tile([P, Dh], F32, tag="dve_junk")
                    nc.vector.scalar_tensor_tensor(
                        out=jd[:], in0=sl[last][:, off : off + Dh], scalar=1.0,
                        in1=half[:], op0=mult, op1=mult,
                        accum_out=dh[:, j : j + 1],
                    )
                nc.vector.tensor_tensor(out=ttt[:, last : last + 1],
                                        in0=th[:, 0:1], in1=th[:, 1:2],
                                        op=mybir.AluOpType.add)
                nc.vector.tensor_tensor(out=dott[:, last : last + 1],
                                        in0=dh[:, 0:1], in1=dh[:, 1:2],
                                        op=mybir.AluOpType.add)

            # epilogue on [P, C]: out = dot / max(sqrt(ss*tt), eps) * mask
            d2 = small.tile([P, C], F32)
            nc.vector.tensor_tensor(out=d2[:], in0=sst[:], in1=ttt[:], op=mult)
            d2c = small.tile([P, C], F32)
            nc.vector.tensor_scalar_max(out=d2c[:], in0=d2[:], scalar1=1e-16)
            den = small.tile([P, C], F32)
            nc.scalar.sqrt(out=den[:], in_=d2c[:])
            rden = small.tile([P, C], F32)
            nc.vector.reciprocal(out=rden[:], in_=den[:])
            cost = small.tile([P, C], F32)
            nc.vector.tensor_tensor(out=cost[:], in0=dott[:], in1=rden[:], op=mult)
            outt = small.tile([P, C], F32)
            nc.vector.tensor_tensor(out=outt[:], in0=cost[:], in1=maskt[:], op=mult)
            out_eng = nc.sync if out_engine == "sync" else nc.gpsimd
            out_eng.dma_start(out=out_d[:], in_=outt[:])
    return nc


def _split_multiwaits(nc: bass.Bass) -> bass.Bass:
    """Hoist all-but-one sync waits onto standalone InstEventSemaphore
    instructions. The walrus build in this container caps the sync-wait
    slots it can encode per instruction (Tile's tail drain carries 14),
    so multi-wait instructions fail codegen with 'Too many sync wait
    commands'. Semantics are identical: the engine sequencer stalls on
    the hoisted waits immediately before the original instruction."""
    for f in nc.m.functions:
        for bb in f.blocks:
            new = []
            changed = False
            for ins in bb.instructions:
                si = ins.sync_info
                waits = (si.on_wait or []) if si else []
                if len(waits) > 1:
                    for k, w in enumerate(waits[:-1]):
                        new.append(mybir.InstEventSemaphore(
                            name=f"{ins.name}-w{k}",
                            engine=ins.engine,
                            ins=[], outs=[],
                            sync_info=mybir.SyncInfo(on_wait=[w], on_update=[]),
                        ))
                    si.on_wait = [waits[-1]]
                    ins.sync_info = si
                    changed = True
                new.append(ins)
            if changed:
                bb.instructions = new
    return nc


_CACHE: dict = {}


def _nc() -> bass.Bass:
    if "nc" not in _CACHE:
        _CACHE["nc"] = _split_multiwaits(build_nc())
    return _CACHE["nc"]


def prepare_in_maps(src_features, tgt_features, src_kps, tgt_kps, valid_mask,
                    patch_size):
    src_features = np.ascontiguousarray(np.asarray(src_features, dtype=np.float32))
    tgt_features = np.ascontiguousarray(np.asarray(tgt_features, dtype=np.float32))
    ps = int(np.asarray(patch_size).reshape(-1)[0])
    sp = np.asarray(src_kps).astype(np.int64) // ps
    tp = np.asarray(tgt_kps).astype(np.int64) // ps
    sx = np.clip(sp[..., 0], 0, W - 1)
    sy = np.clip(sp[..., 1], 0, H - 1)
    tx = np.clip(tp[..., 0], 0, W - 1)
    ty = np.clip(tp[..., 1], 0, H - 1)
    srow = sy * W + sx            # (B, N) row within a batch's H*W block
    trow = ty * W + tx
    mask_f = np.asarray(valid_mask).astype(np.float32)

    boff = np.arange(BPC)[:, None] * (H * W)
    in_maps = []
    for i in range(M):
        b0 = i * BPC
        sflat = (boff + srow[b0 : b0 + BPC]).reshape(KPC)
        tflat = (boff + trow[b0 : b0 + BPC]).reshape(KPC)
        mflat = mask_f[b0 : b0 + BPC].reshape(KPC)
        # device layout [p, c] <-> keypoint k = c*P + p
        meta = np.empty((P, 12), np.int32)
        meta[:, 0:4] = sflat.reshape(C, P).T
        meta[:, 4:8] = tflat.reshape(C, P).T
        meta[:, 8:12] = mflat.reshape(C, P).T.view(np.int32)
        in_maps.append({
            "src_feat": src_features[b0 : b0 + BPC].reshape(ROWS, D),
            "tgt_feat": tgt_features[b0 : b0 + BPC].reshape(ROWS, D),
            "meta": meta,
        })
    return in_maps


def finalize(core_outs, valid_mask) -> np.float32:
    total_cos = 0.0
    for out in core_outs:
        total_cos += float(np.asarray(out, dtype=np.float64).sum())
    n_valid = float(np.asarray(valid_mask).sum())
    return np.float32((n_valid - total_cos) / max(n_valid, 1.0))


def kernel(src_features, tgt_features, src_kps, tgt_kps, valid_mask, patch_size):
    global LAST_RUN
    in_maps = prepare_in_maps(src_features, tgt_features, src_kps, tgt_kps,
                              valid_mask, patch_size)
    try:
        res = run_bass_kernel_spmd(_nc(), in_maps, list(range(M)))
    except ModuleNotFoundError:
        # BASS_TRACE in the environment routes through NTFF profiling hooks
        # that not every container ships; retry with tracing disabled.
        os.environ["BASS_NEVER_TRACE"] = "1"
        res = run_bass_kernel_spmd(_nc(), in_maps, list(range(M)))
    LAST_RUN = res
    return finalize([r["out"] for r in res.results], valid_mask)

